# revision 6
# baseline (speedup 1.0000x reference)
# SAGAN self-attention block (nn_Attention) on 8 TRN2 NeuronCores.
#
# Reference computation per sample (C=256, H=W=64, HW=4096, C8=32, C2=128):
#   theta = w_theta @ x            (32, 4096)
#   phi   = maxpool2(w_phi @ x)    (32, 1024)
#   g     = maxpool2(w_g @ x)      (128, 1024)
#   attn  = softmax(theta.T @ phi, axis=m)          (4096, 1024)
#   o     = w_final @ (attn @ g.T).T                (256, 4096)
#   y     = sigma * o + x
#
# Sharding: data-parallel over batch B=16 -> 2 samples per core, weights
# replicated. No collectives.
#
# Kernel design (per sample, all matmuls bf16 with fp32 PSUM accumulation):
#  - scores are computed TRANSPOSED (m on partitions, n on free):
#      scores_T = phi.T @ theta  via 4x row-packed K=32 matmuls
#    (tile_position row tiling). phi/theta are produced 4x-replicated across
#    partition groups for free by using host-side 4x-replicated conv weights.
#  - exp on ScalarE, psum->sbuf bf16, no max subtraction (|scores| < 29,
#    fp32/bf16 exp range is fine).
#  - O = g.T @ exp_T (contraction over m via PSUM accumulation), plus
#    ones-matmuls accumulating the softmax denominators r.
#  - per n-tile: r -> scatter to 128 partitions -> reciprocal -> broadcast
#    back; O normalized by 1/r, final matmul with sigma-folded w_final,
#    y = F + x on VectorE.

import sys

sys.path.insert(0, "/opt/trn_rl_repo")

import numpy as np
import ml_dtypes

BF = ml_dtypes.bfloat16

B, C, H, W = 16, 256, 64, 64
HW = H * W            # 4096
C8, C2 = C // 8, C // 2   # 32, 128
M = HW // 4           # 1024 pooled positions
NCORES = 8
SPC = B // NCORES     # samples per core = 2
NT = HW // 512        # 8 n-tiles of 512
NCH = M // 128        # 8 m-chunks of 128

_cached = {}


def _build_graph():
    from contextlib import ExitStack
    from concourse import bacc, bass, mybir, tile

    f32 = mybir.dt.float32
    bf16 = mybir.dt.bfloat16
    Exp = mybir.ActivationFunctionType.Exp
    mx = mybir.AluOpType.max

    nc = bacc.Bacc("TRN2", target_bir_lowering=False, debug=False, num_devices=NCORES)

    # ---- DRAM parameters (per-core shard) ----
    x_d = nc.dram_tensor("x", [SPC, C, HW], f32, kind="ExternalInput").ap()
    # host-prepped bf16 weights:
    #   wth_rep / wph_rep: [2(c-chunk), 128(c), 128(=4 replicas x 32 o)]
    wth_d = nc.dram_tensor("wth_rep", [2, 128, 128], bf16, kind="ExternalInput").ap()
    wph_d = nc.dram_tensor("wph_rep", [2, 128, 128], bf16, kind="ExternalInput").ap()
    #   wg_t: [2(c-chunk), 128(c), 128(d)]
    wg_d = nc.dram_tensor("wg_t", [2, 128, 128], bf16, kind="ExternalInput").ap()
    #   wf_t: [2(o-chunk), 128(d), 128(o)]  (already scaled by sigma)
    wf_d = nc.dram_tensor("wf_t", [2, 128, 128], bf16, kind="ExternalInput").ap()
    ident_d = nc.dram_tensor("ident", [128, 128], bf16, kind="ExternalInput").ap()
    ones_d = nc.dram_tensor("ones", [128, 1], bf16, kind="ExternalInput").ap()
    y_d = nc.dram_tensor("y", [SPC, C, HW], f32, kind="ExternalOutput").ap()

    with tile.TileContext(nc) as tc, ExitStack() as ctx:
        # ---- SBUF pools ----
        consts = ctx.enter_context(tc.tile_pool(name="consts", bufs=1))
        xpool = ctx.enter_context(tc.tile_pool(name="x", bufs=2 * SPC))
        xbpool = ctx.enter_context(tc.tile_pool(name="xb", bufs=4))
        thpool = ctx.enter_context(tc.tile_pool(name="theta", bufs=SPC))
        phpool = ctx.enter_context(tc.tile_pool(name="phi", bufs=SPC))
        gpool = ctx.enter_context(tc.tile_pool(name="g", bufs=SPC))
        gtpool = ctx.enter_context(tc.tile_pool(name="gt", bufs=8 * SPC))
        pwpool = ctx.enter_context(tc.tile_pool(name="poolw", bufs=4))
        exppool = ctx.enter_context(tc.tile_pool(name="exp", bufs=8))
        opool = ctx.enter_context(tc.tile_pool(name="oun", bufs=SPC))
        rpool = ctx.enter_context(tc.tile_pool(name="rtiles", bufs=3))
        ypool = ctx.enter_context(tc.tile_pool(name="y", bufs=4))
        # ---- PSUM pools: 2*2 + 1 + 2 + 1 = 8 banks ----
        scps = ctx.enter_context(tc.tile_pool(name="scps", bufs=2, space="PSUM"))
        ops = ctx.enter_context(tc.tile_pool(name="ops", bufs=1, space="PSUM"))
        rps = ctx.enter_context(tc.tile_pool(name="rps", bufs=2, space="PSUM"))
        fps = ctx.enter_context(tc.tile_pool(name="fps", bufs=1, space="PSUM"))

        # ---- load constants/weights ----
        wth = consts.tile([128, 256], bf16, tag="wth")
        wph = consts.tile([128, 256], bf16, tag="wph")
        wg = consts.tile([128, 256], bf16, tag="wg")
        wf = consts.tile([128, 256], bf16, tag="wf")
        ident = consts.tile([128, 128], bf16, tag="ident")
        ones = consts.tile([128, 1], bf16, tag="ones")
        for sb, dr in ((wth, wth_d), (wph, wph_d), (wg, wg_d), (wf, wf_d)):
            for c2 in range(2):
                nc.sync.dma_start(sb[:, 128 * c2:128 * (c2 + 1)], dr[c2])
        nc.sync.dma_start(ident[:], ident_d[:])
        nc.sync.dma_start(ones[:], ones_d[:])

        def wsl(t, c2):
            return t[:, 128 * c2:128 * (c2 + 1)]

        for s in range(SPC):
            # ================= Phase A: projections =================
            x_sb = [xpool.tile([128, HW], f32, tag="x", name=f"x_sb{s}_{c}") for c in range(2)]
            for c2 in range(2):
                nc.sync.dma_start(x_sb[c2][:], x_d[s, 128 * c2:128 * (c2 + 1), :])

            theta = thpool.tile([128, HW], bf16, tag="theta")
            phi = phpool.tile([128, M], bf16, tag="phi")
            g_sb = gpool.tile([128, M], bf16, tag="g")

            for nt in range(NT):
                nsl = slice(512 * nt, 512 * (nt + 1))
                xb = [xbpool.tile([128, 512], bf16, tag="xb", name=f"xb{s}_{nt}_{c}") for c in range(2)]
                for c2 in range(2):
                    nc.gpsimd.tensor_copy(xb[c2][:], x_sb[c2][:, nsl])

                th_ps = scps.tile([128, 512], f32, tag="sc")
                ph_ps = scps.tile([128, 512], f32, tag="sc")
                g_ps = scps.tile([128, 512], f32, tag="sc")
                for c2 in range(2):
                    nc.tensor.matmul(th_ps[:], wsl(wth, c2), xb[c2][:],
                                     start=(c2 == 0), stop=(c2 == 1))
                for c2 in range(2):
                    nc.tensor.matmul(ph_ps[:], wsl(wph, c2), xb[c2][:],
                                     start=(c2 == 0), stop=(c2 == 1))
                for c2 in range(2):
                    nc.tensor.matmul(g_ps[:], wsl(wg, c2), xb[c2][:],
                                     start=(c2 == 0), stop=(c2 == 1))

                nc.vector.tensor_copy(theta[:, nsl], th_ps[:])

                # maxpool 2x2: psum tile is (128, 8 h, 64 w)
                msl = slice(128 * nt, 128 * (nt + 1))
                for src_ps, dst in ((ph_ps, phi), (g_ps, g_sb)):
                    v = src_ps[:].rearrange("p (h w) -> p h w", h=8)
                    tmp = pwpool.tile([128, 8, 32], f32, tag="poolw")
                    nc.vector.tensor_copy(tmp[:], v[:, :, 0::2])
                    nc.vector.tensor_tensor(tmp[:], tmp[:], v[:, :, 1::2], mx)
                    dv = dst[:, msl].rearrange("p (h w) -> p h w", h=4)
                    nc.vector.tensor_tensor(dv, tmp[:, 0::2, :], tmp[:, 1::2, :], mx)

            # g.T via PE transposes: gT[mu] = g[:, 128mu:128mu+128].T
            gT = [gtpool.tile([128, 128], bf16, tag="gt", name=f"gT{s}_{m}") for m in range(NCH)]
            for mu in range(NCH):
                tp_ps = ops.tile([128, 128], bf16, tag="o")
                nc.tensor.transpose(tp_ps[:], g_sb[:, 128 * mu:128 * (mu + 1)], ident[:])
                nc.vector.tensor_copy(gT[mu][:], tp_ps[:])

            # ================= Phase B: attention, per n-tile =================
            o_un = opool.tile([128, HW], bf16, tag="oun")

            for nt in range(NT):
                nsl = slice(512 * nt, 512 * (nt + 1))
                exp_t = []
                for j in range(4):  # pairs of m-chunks
                    sc_ps = scps.tile([128, 1024], f32, tag="sc")
                    for k in range(2):
                        mu = 2 * j + k
                        r_ = mu % 4
                        nc.tensor.matmul(
                            sc_ps[:, 512 * k:512 * (k + 1)],
                            phi[32 * r_:32 * (r_ + 1), 128 * mu:128 * (mu + 1)],
                            theta[32 * r_:32 * (r_ + 1), nsl],
                            start=True, stop=True,
                            tile_position=(32 * r_, 0),
                        )
                    et = exppool.tile([128, 1024], bf16, tag="exp")
                    nc.scalar.activation(et[:], sc_ps[:], Exp)
                    exp_t.append(et)

                o_ps = ops.tile([128, 512], f32, tag="o")
                r_ps = rps.tile([128, 512], f32, tag="r")
                for mu in range(NCH):
                    esl = exp_t[mu // 2][:, 512 * (mu % 2):512 * (mu % 2 + 1)]
                    nc.tensor.matmul(o_ps[:], gT[mu][:], esl,
                                     start=(mu == 0), stop=(mu == NCH - 1))
                    nc.tensor.matmul(r_ps[0:1, :], ones[:], esl,
                                     start=(mu == 0), stop=(mu == NCH - 1))
                nc.vector.tensor_copy(o_un[:, nsl], o_ps[:])

                # ---- softmax denominators: reciprocal on 128 partitions ----
                rf1 = rpool.tile([1, 512], f32, tag="rf1")
                nc.vector.tensor_copy(rf1[:], r_ps[0:1, :])
                rsq = rpool.tile([128, 4], f32, tag="rsq")
                nc.sync.dma_start(rsq[:], rf1[:])       # scatter: p <- 4p..4p+4
                risq = rpool.tile([128, 4], f32, tag="risq")
                nc.vector.reciprocal(risq[:], rsq[:])
                risb = rpool.tile([128, 4], bf16, tag="risb")
                nc.vector.tensor_copy(risb[:], risq[:])
                rf2 = rpool.tile([1, 512], bf16, tag="rf2")
                nc.sync.dma_start(rf2[:], risb[:])      # gather back to flat
                rb = rpool.tile([128, 512], bf16, tag="rb")
                s_ = rf2[0:1, :]
                s_b = bass.AP(s_.tensor, s_.offset, [[512, 1], [0, 128], [1, 512]])
                nc.sync.dma_start(rb[:], s_b)

                # ---- normalize O, final matmul, y = F + x ----
                nc.vector.tensor_mul(o_un[:, nsl], o_un[:, nsl], rb[:])
                for oc in range(2):
                    f_ps = fps.tile([128, 512], f32, tag="f")
                    nc.tensor.matmul(f_ps[:], wsl(wf, oc), o_un[:, nsl],
                                     start=True, stop=True)
                    y_t = ypool.tile([128, 512], f32, tag="y")
                    nc.vector.tensor_add(y_t[:], f_ps[:], x_sb[oc][:, nsl])
                    nc.sync.dma_start(y_d[s, 128 * oc:128 * (oc + 1), nsl], y_t[:])

    nc.compile()
    return nc


def _prep_consts(w_theta, w_phi, w_g, w_final, sigma):
    def rep4(w):  # (32, 256) -> [2, 128, 128] = c-chunks of w.T tiled 4x
        wt = np.asarray(w).T.astype(BF)  # (256, 32)
        out = np.empty((2, 128, 128), dtype=BF)
        for c2 in range(2):
            out[c2] = np.tile(wt[128 * c2:128 * (c2 + 1)], (1, 4))
        return out

    wth = rep4(w_theta)
    wph = rep4(w_phi)
    wgt = np.ascontiguousarray(
        np.asarray(w_g).T.astype(BF).reshape(2, 128, 128))
    wf = (np.float32(sigma) * np.asarray(w_final)).T.astype(BF)  # (128, 256)
    wft = np.ascontiguousarray(wf.reshape(128, 2, 128).transpose(1, 0, 2))
    ident = np.eye(128, dtype=BF)
    ones = np.ones((128, 1), dtype=BF)
    return dict(wth_rep=wth, wph_rep=wph, wg_t=wgt, wf_t=wft,
                ident=ident, ones=ones)


def make_in_maps(x, w_theta, w_phi, w_g, w_final, sigma):
    consts = _prep_consts(w_theta, w_phi, w_g, w_final, sigma)
    xf = np.ascontiguousarray(np.asarray(x).reshape(B, C, HW).astype(np.float32))
    in_maps = []
    for core in range(NCORES):
        m = {"x": xf[SPC * core:SPC * (core + 1)]}
        m.update(consts)
        in_maps.append(m)
    return in_maps


def get_graph():
    if "nc" not in _cached:
        _cached["nc"] = _build_graph()
    return _cached["nc"]


def kernel(**inputs):
    from concourse.bass_utils import run_bass_kernel_spmd

    nc = get_graph()
    in_maps = make_in_maps(**inputs)
    res = run_bass_kernel_spmd(nc, in_maps, core_ids=list(range(NCORES)))
    y = np.concatenate([r["y"] for r in res.results], axis=0)
    return y.reshape(B, C, H, W).astype(np.float32)


if __name__ == "__main__":
    nc = get_graph()
    print("graph built and compiled OK")
